# revision 11
# baseline (speedup 1.0000x reference)
"""Trainium2 Bass kernel for the span-search problem (nn_DCR_21285857919673).

Data-parallel over batch: 32 batches / 8 cores = 4 per core. The host ships
seq pre-transposed ([h, token]), compacted to the valid token span
(sep0+1 .. sep1), and split losslessly into an fp16 hi/lo pair (same 4B/elem
of DMA as fp32, but PE fp16 matmuls run 4x faster than fp32). Per core the
4 batches are sorted into width slots and processed as 2 pairs so each
PSUM-accumulating matmul streams two batches' tokens at once.

Per pair, per h-chunk (8 chunks of 128):
  PE:  d1,d2 via 3 fp16 matmul passes (hi*qhi + hi*qlo + lo*qhi) -> fp32 PSUM
       (error ~2^-22: full-fp32 quality for the argmax ties), in 512-col
       bank-aligned pieces; n2 via fp32r ones-matmuls over Act-squared hi
       (n2 only needs ~5e-5 relative accuracy -- it enters through
       sqrt + ratio -- so the tf32-grade fp32r path is safe there).
Then d/n2 rows go to DRAM scratch and the banded window stage
(overlapping-AP gathers, masked max / first-argmax with a Newton-corrected
division) runs per slot, split across DVE/Pool/Act.
"""
import sys

sys.path.insert(0, "/opt/trn_rl_repo")

import numpy as np

import concourse.bass as bass
import concourse.bacc as bacc
import concourse.bass_isa as bass_isa
import concourse.mybir as mybir
import concourse.tile as tile
from concourse.alu_op_type import AluOpType
from concourse.bass_utils import run_bass_kernel_spmd

F32 = mybir.dt.float32
F32R = mybir.dt.float32r
F16 = mybir.dt.float16
I32 = mybir.dt.int32
U8 = mybir.dt.uint8

B = 32
S = 1024
H = 1024
L = 32
NC = H // 128          # h chunks
NCORES = 8
B_PER_CORE = 4
NEG = -10000.0
PAD_VAL = 0.25         # pad token value -> n2 = 64, keeps denom well-formed

_cache = {}


def _pieces_bank(w):
    """Bank-aligned <=512 pieces of [0, w): [(off, len), ...]."""
    out = []
    off = 0
    while off < w:
        out.append((off, min(512, w - off)))
        off += 512
    return out


def _pieces_balanced(w):
    """Even-length pieces of [0, w), each <=512 and (when possible) >=256.

    fp32r matmuls require an even number of output columns and run at
    full rate only when the moving dim is >=256.
    """
    assert w % 2 == 0
    n = max(1, (w + 511) // 512)
    base = (w // n) & ~1
    out = []
    off = 0
    for i in range(n):
        ln = base if i < n - 1 else w - off
        out.append((off, ln))
        off += ln
    assert all(ln % 2 == 0 and ln <= 512 for _, ln in out)
    return out


def _build(W, NT, pairs):
    """W: slot widths [4], NT: ceil(W/128) [4], pairs: [(slotA, slotB), ...]."""
    NT_MAX = max(NT)
    SP = NT_MAX * 128 + 64      # scratch row pitch
    nc = bacc.Bacc("TRN2", target_bir_lowering=False, debug=False)

    his, los = [], []
    for p, (a, b) in enumerate(pairs):
        Wp = W[a] + W[b]
        his.append(nc.dram_tensor(f"hi{p}", [NC, 128, Wp], F16, kind="ExternalInput").ap())
        los.append(nc.dram_tensor(f"lo{p}", [NC, 128, Wp], F16, kind="ExternalInput").ap())
    qh_in = nc.dram_tensor("qh", [128, NC * 2 * 4], F16, kind="ExternalInput").ap()
    ql_in = nc.dram_tensor("ql", [128, NC * 2 * 4], F16, kind="ExternalInput").ap()
    qcat_in = nc.dram_tensor("qcat", [128, 4 * 16], F32, kind="ExternalInput").ap()
    ones_in = nc.dram_tensor("ones", [128, 1], F32R, kind="ExternalInput").ap()
    riota_in = nc.dram_tensor("riota", [128, NT_MAX * L], F32, kind="ExternalInput").ap()
    vmj_in, vmi_in, cconst_in = [], [], []
    for k in range(4):
        vmj_in.append(nc.dram_tensor(f"vmj{k}", [128, NT[k] * L], U8, kind="ExternalInput").ap())
        vmi_in.append(nc.dram_tensor(f"vmi{k}", [128, NT[k]], U8, kind="ExternalInput").ap())
        cconst_in.append(nc.dram_tensor(f"cconst{k}", [128, NT[k]], F32, kind="ExternalInput").ap())

    mv_out = nc.dram_tensor("mv_out", [4, NT_MAX * 128], F32, kind="ExternalOutput").ap()
    ei_out = nc.dram_tensor("ei_out", [4, NT_MAX * 128], I32, kind="ExternalOutput").ap()
    scratch = nc.dram_tensor("scratch", [4, 3, SP], F32).ap()

    with tile.TileContext(nc) as tc:
        with (
            tc.tile_pool(name="consts", bufs=1) as consts,
            tc.tile_pool(name="seqp", bufs=3) as seqp,
            tc.tile_pool(name="sqp", bufs=3) as sqp,
            tc.tile_pool(name="rows", bufs=2) as rows_p,
            tc.tile_pool(name="band", bufs=2) as band_p,
            tc.tile_pool(name="pd", bufs=1, space="PSUM") as pd,
            tc.tile_pool(name="pn", bufs=1, space="PSUM") as pn,
        ):
            # ---- constants ----
            c_qh = consts.tile([128, NC * 2 * 4], F16, tag="qh")
            nc.sync.dma_start(c_qh[:], qh_in)
            c_ql = consts.tile([128, NC * 2 * 4], F16, tag="ql")
            nc.sync.dma_start(c_ql[:], ql_in)
            c_qcat = consts.tile([128, 4 * 16], F32, tag="qcat")
            nc.sync.dma_start(c_qcat[:], qcat_in)
            c_ones = consts.tile([128, 1], F32R, tag="ones")
            nc.sync.dma_start(c_ones[:], ones_in)
            c_riota = consts.tile([128, NT_MAX * L], F32, tag="riota")
            nc.sync.dma_start(c_riota[:], riota_in)
            c_vmj, c_vmi, c_cc = [], [], []
            for k in range(4):
                t = consts.tile([128, NT[k] * L], U8, tag=f"vmj{k}")
                nc.sync.dma_start(t[:], vmj_in[k])
                c_vmj.append(t)
                t = consts.tile([128, NT[k]], U8, tag=f"vmi{k}")
                nc.sync.dma_start(t[:], vmi_in[k])
                c_vmi.append(t)
                t = consts.tile([128, NT[k]], F32, tag=f"cc{k}")
                nc.sync.dma_start(t[:], cconst_in[k])
                c_cc.append(t)
            c_negL = consts.tile([128, NT_MAX * L], F32, tag="negL")
            nc.vector.memset(c_negL[:], NEG)
            c_neg1 = consts.tile([128, NT_MAX], F32, tag="neg1")
            nc.vector.memset(c_neg1[:], -1.0)
            c_onerow = consts.tile([1, 192], F32, tag="onerow")
            nc.vector.memset(c_onerow[:], 1.0)
            # scratch pad rows (region [W_k, NT_k*128+32) must be finite)
            for k in range(4):
                pad = NT[k] * 128 + 32 - W[k]
                if pad > 0:
                    for r in range(3):
                        nc.scalar.dma_start(
                            bass.AP(scratch.tensor, (k * 3 + r) * SP + W[k], [[1, 1], [1, pad]]),
                            c_onerow[0:1, 0:pad])

            # ---- qn2 -> rsqall [128, 4] (1/sqrt(|q1|^2+|q2|^2) per slot) ----
            qpart = consts.tile([128, 4], F32, tag="qpart")
            qtrash = consts.tile([128, 16], F32, tag="qtrash")
            for k in range(4):
                nc.scalar.activation(qtrash[:], c_qcat[:, k * 16:(k + 1) * 16],
                                     mybir.ActivationFunctionType.Square,
                                     accum_out=qpart[:, k:k + 1])
            qn2all = consts.tile([128, 4], F32, tag="qn2all")
            nc.gpsimd.partition_all_reduce(qn2all[:], qpart[:], 128, bass_isa.ReduceOp.add)
            qsq = consts.tile([128, 4], F32, tag="qsq")
            nc.scalar.sqrt(qsq[:], qn2all[:])
            rsqall = consts.tile([128, 4], F32, tag="rsqall")
            nc.vector.reciprocal(rsqall[:], qsq[:])

            def bcast_l(tile_, nt):
                a = tile_[:]
                return bass.AP(tile_.tensor, a.offset,
                               [[a.ap[0][0], 128], [a.ap[-1][0], nt], [0, L]])

            def win_col(tile_, nt):
                a = tile_[:]
                return bass.AP(tile_.tensor, a.offset,
                               [[a.ap[0][0], 128], [a.ap[1][0], nt], [0, L]])

            # ================= main pair loop =================
            for p, (ka, kb) in enumerate(pairs):
                Wa, Wb = W[ka], W[kb]
                Wp = Wa + Wb
                dpieces = _pieces_bank(Wp)
                npieces = _pieces_balanced(Wp)
                # fp32r matmul outputs must sit at PSUM partition 0 -> one
                # single-bank psum tile per n2 piece
                dps = pd.tile([4, Wp], F32, tag="dps")
                ntile = [pn.tile([1, 512], F32, tag=f"n{j}", name=f"n{j}_{p}")
                         for j in range(len(npieces))]
                sq_tiles = []
                for c in range(NC):
                    hi_c = seqp.tile([128, Wp], F16, tag="hi")
                    nc.sync.dma_start(hi_c[:], his[p][c])
                    lo_c = seqp.tile([128, Wp], F16, tag="lo")
                    nc.sync.dma_start(lo_c[:], los[p][c])
                    qh_sl = c_qh[:, (c * 2 + p) * 4:(c * 2 + p) * 4 + 4]
                    ql_sl = c_ql[:, (c * 2 + p) * 4:(c * 2 + p) * 4 + 4]
                    for off, ln in dpieces:
                        nc.tensor.matmul(dps[:, off:off + ln], lhsT=qh_sl,
                                         rhs=hi_c[:, off:off + ln],
                                         start=(c == 0), stop=False)
                        nc.tensor.matmul(dps[:, off:off + ln], lhsT=ql_sl,
                                         rhs=hi_c[:, off:off + ln],
                                         start=False, stop=False)
                        nc.tensor.matmul(dps[:, off:off + ln], lhsT=qh_sl,
                                         rhs=lo_c[:, off:off + ln],
                                         start=False, stop=(c == NC - 1))
                    sq_c = sqp.tile([128, Wp], F32R, tag="sq")
                    nc.scalar.activation(sq_c[:], hi_c[:],
                                         mybir.ActivationFunctionType.Square)
                    sq_tiles.append(sq_c)
                    # n2 ones-matmuls for the previous chunk (sq lags the DMA)
                    for cc2 in ([c - 1] if c > 0 else []) + ([c] if c == NC - 1 else []):
                        for j, (off, ln) in enumerate(npieces):
                            nc.tensor.matmul(ntile[j][0:1, 0:ln], lhsT=c_ones[:],
                                             rhs=sq_tiles[cc2][:, off:off + ln],
                                             start=(cc2 == 0), stop=(cc2 == NC - 1))

                # drain d rows and n2 pieces PSUM -> SBUF
                dsb = rows_p.tile([4, Wp], F32, tag="dsb")
                nc.scalar.copy(dsb[:], dps[:])
                n2sb = rows_p.tile([1, Wp], F32, tag="n2sb")
                for j, (off, ln) in enumerate(npieces):
                    nc.vector.tensor_copy(n2sb[0:1, off:off + ln], ntile[j][0:1, 0:ln])

                # scratch writes (rows are linear in compacted token index)
                for k, r0, wlo in ((ka, 0, 0), (kb, 2, Wa)):
                    wk = W[k]
                    nc.scalar.dma_start(
                        bass.AP(scratch.tensor, k * 3 * SP, [[SP, 2], [1, wk]]),
                        dsb[r0:r0 + 2, wlo:wlo + wk])
                    nc.scalar.dma_start(
                        bass.AP(scratch.tensor, (k * 3 + 2) * SP, [[1, 1], [1, wk]]),
                        n2sb[0:1, wlo:wlo + wk])

                # ---- banded stage per slot ----
                for k in (ka, kb):
                    nt = NT[k]
                    soff = k * 3 * SP
                    d1col = band_p.tile([128, nt], F32, tag="d1col")
                    nc.scalar.dma_start(d1col[:], bass.AP(scratch.tensor, soff, [[1, 128], [128, nt]]))
                    d2w = band_p.tile([128, nt, L], F32, tag="d2w")
                    nc.scalar.dma_start(d2w[:], bass.AP(scratch.tensor, soff + SP,
                                                        [[1, 128], [128, nt], [1, L]]))
                    n2w = band_p.tile([128, nt, L], F32, tag="n2w")
                    nc.scalar.dma_start(n2w[:], bass.AP(scratch.tensor, soff + 2 * SP,
                                                        [[1, 128], [128, nt], [1, L]]))

                    numer = band_p.tile([128, nt, L], F32, tag="numer")
                    nc.gpsimd.tensor_tensor(out=numer[:], in0=d2w[:], in1=bcast_l(d1col, nt),
                                            op=AluOpType.add)
                    nsum = band_p.tile([128, nt, L], F32, tag="nsum")
                    nc.vector.tensor_tensor(out=nsum[:], in0=n2w[:], in1=win_col(n2w, nt),
                                            op=AluOpType.add)
                    den = band_p.tile([128, nt, L], F32, tag="den")
                    nc.scalar.sqrt(den[:], nsum[:])
                    rec = band_p.tile([128, nt, L], F32, tag="rec")
                    nc.vector.reciprocal(rec[:], den[:])
                    sim0 = band_p.tile([128, nt, L], F32, tag="sim0")
                    nc.vector.tensor_tensor(out=sim0[:], in0=numer[:], in1=rec[:],
                                            op=AluOpType.mult)
                    # Newton step: sim = sim0 + (numer - sim0*den) * rec
                    nt1 = band_p.tile([128, nt, L], F32, tag="nt1")
                    nc.gpsimd.tensor_tensor(out=nt1[:], in0=sim0[:], in1=den[:],
                                            op=AluOpType.mult)
                    nt2 = band_p.tile([128, nt, L], F32, tag="nt2")
                    nc.gpsimd.tensor_tensor(out=nt2[:], in0=numer[:], in1=nt1[:],
                                            op=AluOpType.subtract)
                    nt3 = band_p.tile([128, nt, L], F32, tag="nt3")
                    nc.gpsimd.tensor_tensor(out=nt3[:], in0=nt2[:], in1=rec[:],
                                            op=AluOpType.mult)
                    sim = band_p.tile([128, nt, L], F32, tag="sim")
                    nc.gpsimd.tensor_tensor(out=sim[:], in0=sim0[:], in1=nt3[:],
                                            op=AluOpType.add)
                    simm = band_p.tile([128, nt, L], F32, tag="simm")
                    nc.vector.select(simm[:], c_vmj[k][:].rearrange("p (c l) -> p c l", l=L),
                                     sim[:], c_negL[:, 0:nt * L].rearrange("p (c l) -> p c l", l=L))
                    maxv = band_p.tile([128, nt], F32, tag="maxv")
                    nc.vector.tensor_reduce(out=maxv[:], in_=simm[:], axis=mybir.AxisListType.X,
                                            op=AluOpType.max)
                    eq = band_p.tile([128, nt, L], F32, tag="eq")
                    nc.vector.tensor_tensor(out=eq[:], in0=simm[:], in1=bcast_l(maxv, nt),
                                            op=AluOpType.is_equal)
                    wt = band_p.tile([128, nt, L], F32, tag="wt")
                    nc.gpsimd.tensor_tensor(out=wt[:], in0=eq[:],
                                            in1=c_riota[:, 0:nt * L].rearrange("p (c l) -> p c l", l=L),
                                            op=AluOpType.mult)
                    mval = band_p.tile([128, nt], F32, tag="mval")
                    nc.vector.tensor_reduce(out=mval[:], in_=wt[:], axis=mybir.AxisListType.X,
                                            op=AluOpType.max)
                    endf = band_p.tile([128, nt], F32, tag="endf")
                    nc.gpsimd.tensor_tensor(out=endf[:], in0=c_cc[k][:], in1=mval[:],
                                            op=AluOpType.subtract)
                    mvs = band_p.tile([128, nt], F32, tag="mvs")
                    nc.vector.tensor_scalar(out=mvs[:], in0=maxv[:], scalar1=rsqall[:, k:k + 1],
                                            scalar2=None, op0=AluOpType.mult)
                    mvf = band_p.tile([128, nt], F32, tag="mvf")
                    nc.vector.select(mvf[:], c_vmi[k][:], mvs[:], c_negL[:, 0:nt])
                    eif = band_p.tile([128, nt], F32, tag="eif")
                    nc.vector.select(eif[:], c_vmi[k][:], endf[:], c_neg1[:, 0:nt])
                    eii = band_p.tile([128, nt], I32, tag="eii")
                    nc.vector.tensor_copy(eii[:], eif[:])

                    # outputs: tile-order contiguous write; host de-permutes
                    nc.scalar.dma_start(
                        bass.AP(mv_out.tensor, k * NT_MAX * 128, [[nt, 128], [1, nt]]), mvf[:])
                    nc.scalar.dma_start(
                        bass.AP(ei_out.tensor, k * NT_MAX * 128, [[nt, 128], [1, nt]]), eii[:])

    nc.compile()
    return nc


def _prep_core(seq, idx, order_c, W, NT, pairs):
    """Host-side input prep for one core.

    seq: full [B, S, H] f32; order_c: global batch index per slot (len 4).
    """
    NT_MAX = max(NT)
    p128 = np.arange(128)
    qh = np.zeros((128, NC * 2 * 4), np.float16)
    ql = np.zeros((128, NC * 2 * 4), np.float16)
    qcat = np.zeros((128, 4 * 16), np.float32)
    im = {}
    spans = {}
    for p, (ka, kb) in enumerate(pairs):
        Wp = W[ka] + W[kb]
        hi_p = np.full((NC, 128, Wp), PAD_VAL, np.float16)
        lo_p = np.zeros((NC, 128, Wp), np.float16)
        for k, wlo in ((ka, 0), (kb, W[ka])):
            b = order_c[k]
            sep0, sep1 = int(idx[b, 0]), int(idx[b, 1])
            span = max(0, sep1 - sep0 - 1)
            spans[k] = span
            x = np.ascontiguousarray(seq[b, sep0 + 1:sep0 + 1 + span, :].T)  # [H, span]
            xh = x.astype(np.float16)
            xl = (x - xh.astype(np.float32)).astype(np.float16)
            hi_p[:, :, wlo:wlo + span] = xh.reshape(NC, 128, span)
            lo_p[:, :, wlo:wlo + span] = xl.reshape(NC, 128, span)
            q1 = seq[b, 1, :]
            q2 = seq[b, sep0 - 1, :]
            q1h, q2h = q1.astype(np.float16), q2.astype(np.float16)
            q1l = (q1 - q1h.astype(np.float32)).astype(np.float16)
            q2l = (q2 - q2h.astype(np.float32)).astype(np.float16)
            j0 = 0 if k == ka else 2
            for c in range(NC):
                base = (c * 2 + p) * 4 + j0
                sl = slice(c * 128, (c + 1) * 128)
                qh[:, base] = q1h[sl]
                qh[:, base + 1] = q2h[sl]
                ql[:, base] = q1l[sl]
                ql[:, base + 1] = q2l[sl]
            qcat[:, k * 16:k * 16 + 8] = q1.reshape(128, 8, order="F")
            qcat[:, k * 16 + 8:k * 16 + 16] = q2.reshape(128, 8, order="F")
        im[f"hi{p}"] = hi_p
        im[f"lo{p}"] = lo_p
    for k in range(4):
        b = order_c[k]
        sep0 = int(idx[b, 0])
        span = spans[k]
        nt = NT[k]
        i_comp = p128[:, None] + 128 * np.arange(nt)[None, :]            # [128, nt]
        jv = i_comp[:, :, None] + np.arange(L)[None, None, :]            # [128, nt, L]
        im[f"vmj{k}"] = (jv < span).astype(np.uint8).reshape(128, nt * L)
        im[f"vmi{k}"] = (i_comp < span).astype(np.uint8)
        im[f"cconst{k}"] = (sep0 + 1 + i_comp + L).astype(np.float32)
    im["qh"] = qh
    im["ql"] = ql
    im["qcat"] = qcat
    im["ones"] = np.ones((128, 1), np.float32)
    im["riota"] = np.broadcast_to((L - np.arange(L))[None, None, :],
                                  (128, NT_MAX, L)).reshape(128, NT_MAX * L).astype(np.float32)
    return im, spans


def kernel(sequence_outputs, idxs, max_ans_len):
    seq = np.asarray(sequence_outputs, dtype=np.float32)
    idx = np.asarray(idxs).astype(np.int64)
    assert int(max_ans_len) == L and seq.shape == (B, S, H)

    spans_all = np.maximum(idx[:, 1] - idx[:, 0] - 1, 0)
    order = np.argsort(-spans_all, kind="stable")          # global desc
    # slot k on core c processes global batch order[k*8 + c]
    W = [max(2, (int(spans_all[order[k * NCORES]]) + 1) & ~1) for k in range(4)]
    NT = [(w + 127) // 128 for w in W]
    pairs = [(0, 3), (1, 2)]

    key = (tuple(W),)
    if key not in _cache:
        _cache[key] = _build(W, NT, pairs)
    nc = _cache[key]

    NT_MAX = max(NT)
    in_maps, span_list = [], []
    for c in range(NCORES):
        order_c = [int(order[k * NCORES + c]) for k in range(4)]
        im, spans = _prep_core(seq, idx, order_c, W, NT, pairs)
        in_maps.append(im)
        span_list.append((order_c, spans))

    res = run_bass_kernel_spmd(nc, in_maps, core_ids=list(range(NCORES))).results

    mv = np.full((B, S), NEG, np.float32)
    ei = np.full((B, S), -1, np.int32)
    for c in range(NCORES):
        order_c, spans = span_list[c]
        for k in range(4):
            b = order_c[k]
            sep0 = int(idx[b, 0])
            span = spans[k]
            if span <= 0:
                continue
            nt = NT[k]
            # device wrote [128, nt] tiles contiguously; token i = p + 128t
            mvd = res[c]["mv_out"][k, 0:128 * nt].reshape(128, nt).T.ravel()
            eid = res[c]["ei_out"][k, 0:128 * nt].reshape(128, nt).T.ravel()
            mv[b, sep0 + 1:sep0 + 1 + span] = mvd[0:span]
            ei[b, sep0 + 1:sep0 + 1 + span] = eid[0:span]
    return mv, ei


# revision 18
# speedup vs baseline: 1.0154x; 1.0154x over previous
"""Trainium2 Bass kernel for the span-search problem (nn_DCR_21285857919673).

Data-parallel over batch: 32 batches / 8 cores = 4 per core. The host ships
seq pre-transposed ([h, token]), compacted to the valid token span
(sep0+1 .. sep1), and split losslessly into an fp16 hi/lo pair (same 4B/elem
of DMA as fp32, but PE fp16 matmuls run 4x faster than fp32). Per core the
4 batches are sorted into width slots and processed as 2 pairs so each
PSUM-accumulating matmul streams two batches' tokens at once.

Per pair, per h-chunk (8 chunks of 128):
  PE:  d1,d2 via 3 fp16 matmul passes (hi*qhi + hi*qlo + lo*qhi) -> fp32 PSUM
       (error ~2^-22: full-fp32 quality for the argmax ties), in 512-col
       bank-aligned pieces; n2 via fp32r ones-matmuls over Act-squared hi
       (n2 only needs ~5e-5 relative accuracy -- it enters through
       sqrt + ratio -- so the tf32-grade fp32r path is safe there).
Then d/n2 rows go to DRAM scratch and the banded window stage
(overlapping-AP gathers, masked max / first-argmax with a Newton-corrected
division) runs per slot, split across DVE/Pool/Act. Pair k's banded stage
is emitted inside pair k+1's chunk loop so it overlaps the next pair's
compute; DMA instruction count is kept low (HWDGE prep is ~0.6us each).
"""
import sys

sys.path.insert(0, "/opt/trn_rl_repo")

import numpy as np

import concourse.bass as bass
import concourse.bacc as bacc
import concourse.bass_isa as bass_isa
import concourse.mybir as mybir
import concourse.tile as tile
from concourse.alu_op_type import AluOpType
from concourse.bass_utils import run_bass_kernel_spmd

F32 = mybir.dt.float32
F32R = mybir.dt.float32r
F16 = mybir.dt.float16
I32 = mybir.dt.int32
U8 = mybir.dt.uint8

B = 32
S = 1024
H = 1024
L = 32
NC = H // 128          # h chunks
NCORES = 8
NEG = -10000.0
PAD_VAL = 0.25

CHUNK_GROUPS = [[0], [1, 2], [3, 4], [5, 6], [7]]

_cache = {}


def _pieces_bank(w):
    """Bank-aligned <=512 pieces of [0, w)."""
    return [(off, min(512, w - off)) for off in range(0, w, 512)]


def _pieces_balanced(w):
    """Even-length pieces of [0, w), each <=512 (fp32r needs even cols,
    and >=256 cols for full rate)."""
    assert w % 2 == 0
    n = max(1, (w + 511) // 512)
    base = (w // n) & ~1
    out = []
    off = 0
    for i in range(n):
        ln = base if i < n - 1 else w - off
        out.append((off, ln))
        off += ln
    assert all(ln % 2 == 0 and ln <= 512 for _, ln in out)
    return out


def _build(W, NT, pairs):
    """W: slot widths [4] (even), NT: ceil(W/128) [4], pairs: [(a, b), ...]."""
    NT_MAX = max(NT)
    SP = NT_MAX * 128 + 64      # scratch row pitch
    LU = sum(NT[k] * L for k in range(4))
    LI = sum(NT[k] for k in range(4))
    nc = bacc.Bacc("TRN2", target_bir_lowering=False, debug=False)

    # partition-major so multi-chunk DMA slices match SBUF iteration order
    hilos = []
    for p, (a, b) in enumerate(pairs):
        Wp = W[a] + W[b]
        hilos.append(nc.dram_tensor(f"hilo{p}", [128, NC * 2 * Wp], F16,
                                    kind="ExternalInput").ap())
    # qhl: qh block then ql block, each [128, NC*2*4]
    qhl_in = nc.dram_tensor("qhl", [128, 2 * NC * 2 * 4], F16, kind="ExternalInput").ap()
    # u8 pack: all vmj (slot-major) then all vmi
    vpack_in = nc.dram_tensor("vpack", [128, LU + LI], U8, kind="ExternalInput").ap()
    # f32 pack: riota | all cconst | qcat
    fpack_in = nc.dram_tensor("fpack", [128, NT_MAX * L + LI + 4 * 16], F32,
                              kind="ExternalInput").ap()
    ones_in = nc.dram_tensor("ones", [128, 2], F32R, kind="ExternalInput").ap()

    mvei_out = nc.dram_tensor("mvei", [4, 2 * NT_MAX * 128], F32, kind="ExternalOutput").ap()
    scratch = nc.dram_tensor("scratch", [4, 3, SP], F32).ap()

    with tile.TileContext(nc) as tc:
        with (
            tc.tile_pool(name="consts", bufs=1) as consts,
            tc.tile_pool(name="seqp", bufs=3) as seqp,
            tc.tile_pool(name="sqp", bufs=3) as sqp,
            tc.tile_pool(name="rows", bufs=2) as rows_p,
            tc.tile_pool(name="band", bufs=2) as band_p,
            tc.tile_pool(name="pd", bufs=1, space="PSUM") as pd,
            tc.tile_pool(name="pn", bufs=1, space="PSUM") as pn,
        ):
            # ---------- pair-0 chunk streams first (PE warmup path) ----------
            pair_tiles = {}

            def emit_chunk_dmas(p, Wp):
                tiles = []
                for g in CHUNK_GROUPS:
                    t = seqp.tile([128, len(g) * 2 * Wp], F16, tag=f"hilo{len(g)}",
                                  name=f"hilo_p{p}_g{g[0]}")
                    nc.sync.dma_start(t[:], hilos[p][:, g[0] * 2 * Wp:(g[-1] + 1) * 2 * Wp])
                    for gi, c in enumerate(g):
                        tiles.append((t, gi))
                return tiles

            Wps = [W[a] + W[b] for a, b in pairs]
            pair_tiles[0] = emit_chunk_dmas(0, Wps[0])

            # ---------- consts ----------
            c_qhl = consts.tile([128, 2 * NC * 2 * 4], F16, tag="qhl")
            nc.sync.dma_start(c_qhl[:], qhl_in)
            c_vpack = consts.tile([128, LU + LI], U8, tag="vpack")
            nc.sync.dma_start(c_vpack[:], vpack_in)
            c_fpack = consts.tile([128, NT_MAX * L + LI + 4 * 16], F32, tag="fpack")
            nc.sync.dma_start(c_fpack[:], fpack_in)
            c_ones = consts.tile([128, 2], F32R, tag="ones")
            nc.sync.dma_start(c_ones[:], ones_in)

            pair_tiles[1] = emit_chunk_dmas(1, Wps[1])

            # const views
            off_u = [0]
            for k in range(4):
                off_u.append(off_u[-1] + NT[k] * L)
            vmj = [c_vpack[:, off_u[k]:off_u[k + 1]] for k in range(4)]
            off_i = LU
            vmi = []
            for k in range(4):
                vmi.append(c_vpack[:, off_i:off_i + NT[k]])
                off_i += NT[k]
            riota = c_fpack[:, 0:NT_MAX * L]
            off_f = NT_MAX * L
            cconst = []
            for k in range(4):
                cconst.append(c_fpack[:, off_f:off_f + NT[k]])
                off_f += NT[k]
            qcat = c_fpack[:, off_f:off_f + 4 * 16]

            c_negL = consts.tile([128, NT_MAX * L], F32, tag="negL")
            nc.vector.memset(c_negL[:], NEG)
            c_neg1 = consts.tile([128, NT_MAX], F32, tag="neg1")
            nc.vector.memset(c_neg1[:], -1.0)

            qpart = consts.tile([128, 4], F32, tag="qpart")
            qtrash = consts.tile([128, 16], F32, tag="qtrash")
            qn2all = consts.tile([128, 4], F32, tag="qn2all")
            qsq = consts.tile([128, 4], F32, tag="qsq")
            rsqall = consts.tile([128, 4], F32, tag="rsqall")

            def emit_qn2():
                for k in range(4):
                    nc.scalar.activation(qtrash[:], qcat[:, k * 16:(k + 1) * 16],
                                         mybir.ActivationFunctionType.Square,
                                         accum_out=qpart[:, k:k + 1])
                nc.gpsimd.partition_all_reduce(qn2all[:], qpart[:], 128,
                                               bass_isa.ReduceOp.add)
                nc.scalar.sqrt(qsq[:], qn2all[:])
                nc.vector.reciprocal(rsqall[:], qsq[:])

            def bcast_l(ap_col, nt):
                return bass.AP(ap_col.tensor, ap_col.offset,
                               [[ap_col.ap[0][0], 128], [ap_col.ap[-1][0], nt], [0, L]])

            def win_col(tile_, nt):
                a = tile_[:]
                return bass.AP(tile_.tensor, a.offset,
                               [[a.ap[0][0], 128], [a.ap[1][0], nt], [0, L]])

            def rearr(ap_, nt):
                return bass.AP(ap_.tensor, ap_.offset,
                               [[ap_.ap[0][0], 128], [L, nt], [1, L]])

            # ---------- phase A (matmuls/squares) ----------
            def emit_phase_a(p, banded_mid=None):
                ka, kb = pairs[p]
                Wp = Wps[p]
                dpieces = _pieces_bank(Wp)
                npieces = _pieces_balanced(Wp)
                dps = pd.tile([4, Wp], F32, tag="dps", name=f"dps{p}")
                ntile = [pn.tile([2, 512], F32, tag=f"n{j}", name=f"n{j}_{p}")
                         for j in range(len(npieces))]
                sq_tiles = []
                for c in range(NC):
                    t, gi = pair_tiles[p][c]
                    hi_c = t[:, gi * 2 * Wp: gi * 2 * Wp + Wp]
                    lo_c = t[:, gi * 2 * Wp + Wp: (gi + 1) * 2 * Wp]
                    qh_sl = c_qhl[:, (c * 2 + p) * 4:(c * 2 + p) * 4 + 4]
                    ql_sl = c_qhl[:, NC * 8 + (c * 2 + p) * 4:NC * 8 + (c * 2 + p) * 4 + 4]
                    for off, ln in dpieces:
                        nc.tensor.matmul(dps[:, off:off + ln], lhsT=qh_sl,
                                         rhs=hi_c[:, off:off + ln],
                                         start=(c == 0), stop=False)
                        nc.tensor.matmul(dps[:, off:off + ln], lhsT=ql_sl,
                                         rhs=hi_c[:, off:off + ln],
                                         start=False, stop=False)
                        nc.tensor.matmul(dps[:, off:off + ln], lhsT=qh_sl,
                                         rhs=lo_c[:, off:off + ln],
                                         start=False, stop=(c == NC - 1))
                    sq_c = sqp.tile([128, Wp], F32R, tag="sq", name=f"sq{p}_{c}")
                    nc.scalar.activation(sq_c[:], hi_c,
                                         mybir.ActivationFunctionType.Square)
                    sq_tiles.append(sq_c)
                    for cc2 in ([c - 1] if c > 0 else []) + ([c] if c == NC - 1 else []):
                        for j, (off, ln) in enumerate(npieces):
                            nc.tensor.matmul(ntile[j][0:1, 0:ln], lhsT=c_ones[:, 0:1],
                                             rhs=sq_tiles[cc2][:, off:off + ln],
                                             start=(cc2 == 0), stop=(cc2 == NC - 1))
                    if p == 0 and c == NC - 1:
                        emit_qn2()
                    if banded_mid is not None and c == 3:
                        banded_mid()
                return dps, ntile, npieces

            # ---------- phase B part 1: drain + scratch ----------
            def emit_drain(p, dps, ntile, npieces):
                ka, kb = pairs[p]
                Wp = Wps[p]
                Wa = W[ka]
                dsb = rows_p.tile([4, Wp + 160], F32, tag="dsb", name=f"dsb{p}")
                nc.scalar.copy(dsb[:, 0:Wp], dps[:])
                n2sb = rows_p.tile([1, Wp + 160], F32, tag="n2sb", name=f"n2sb{p}")
                for j, (off, ln) in enumerate(npieces):
                    nc.vector.tensor_copy(n2sb[0:1, off:off + ln], ntile[j][0:1, 0:ln])
                # scratch rows; width covers the window pad region ([W_k, NT_k*128+32))
                for k, r0, wlo in ((ka, 0, 0), (kb, 2, Wa)):
                    wk = NT[k] * 128 + 32
                    nc.sync.dma_start(
                        bass.AP(scratch.tensor, k * 3 * SP, [[SP, 2], [1, wk]]),
                        dsb[r0:r0 + 2, wlo:wlo + wk])
                    nc.sync.dma_start(
                        bass.AP(scratch.tensor, (k * 3 + 2) * SP, [[1, 1], [1, wk]]),
                        n2sb[0:1, wlo:wlo + wk])

            # ---------- phase B part 2: banded stage for one pair ----------
            def emit_banded(p):
                ka, kb = pairs[p]
                work = []
                for k in (ka, kb):
                    nt = NT[k]
                    soff = k * 3 * SP
                    d1col = band_p.tile([128, nt], F32, tag="d1col", name=f"d1c{k}")
                    nc.sync.dma_start(d1col[:], bass.AP(scratch.tensor, soff,
                                                        [[1, 128], [128, nt]]))
                    d2w = band_p.tile([128, nt, L], F32, tag="d2w", name=f"d2w{k}")
                    nc.sync.dma_start(d2w[:], bass.AP(scratch.tensor, soff + SP,
                                                      [[1, 128], [128, nt], [1, L]]))
                    n2w = band_p.tile([128, nt, L], F32, tag="n2w", name=f"n2w{k}")
                    nc.sync.dma_start(n2w[:], bass.AP(scratch.tensor, soff + 2 * SP,
                                                      [[1, 128], [128, nt], [1, L]]))
                    work.append((k, nt, d1col, d2w, n2w))

                st = {}
                for k, nt, d1col, d2w, n2w in work:
                    numer = band_p.tile([128, nt, L], F32, tag="numer", name=f"nu{k}")
                    nc.gpsimd.tensor_tensor(out=numer[:], in0=d2w[:],
                                            in1=bcast_l(d1col[:], nt), op=AluOpType.add)
                    nsum = band_p.tile([128, nt, L], F32, tag="nsum", name=f"ns{k}")
                    nc.vector.tensor_tensor(out=nsum[:], in0=n2w[:], in1=win_col(n2w, nt),
                                            op=AluOpType.add)
                    st[k] = [numer, nsum]
                for k, nt, d1col, d2w, n2w in work:
                    numer, nsum = st[k]
                    den = band_p.tile([128, nt, L], F32, tag="den", name=f"de{k}")
                    nc.scalar.sqrt(den[:], nsum[:])
                    rec = band_p.tile([128, nt, L], F32, tag="rec", name=f"re{k}")
                    nc.vector.reciprocal(rec[:], den[:])
                    st[k] += [den, rec]
                for k, nt, d1col, d2w, n2w in work:
                    numer, nsum, den, rec = st[k]
                    sim0 = band_p.tile([128, nt, L], F32, tag="sim0", name=f"s0{k}")
                    nc.vector.tensor_tensor(out=sim0[:], in0=numer[:], in1=rec[:],
                                            op=AluOpType.mult)
                    nt1 = band_p.tile([128, nt, L], F32, tag="nt1", name=f"t1{k}")
                    nc.gpsimd.tensor_tensor(out=nt1[:], in0=sim0[:], in1=den[:],
                                            op=AluOpType.mult)
                    nt2 = band_p.tile([128, nt, L], F32, tag="nt2", name=f"t2{k}")
                    nc.gpsimd.tensor_tensor(out=nt2[:], in0=numer[:], in1=nt1[:],
                                            op=AluOpType.subtract)
                    nt3 = band_p.tile([128, nt, L], F32, tag="nt3", name=f"t3{k}")
                    nc.gpsimd.tensor_tensor(out=nt3[:], in0=nt2[:], in1=rec[:],
                                            op=AluOpType.mult)
                    sim = band_p.tile([128, nt, L], F32, tag="sim", name=f"si{k}")
                    nc.gpsimd.tensor_tensor(out=sim[:], in0=sim0[:], in1=nt3[:],
                                            op=AluOpType.add)
                    simm = band_p.tile([128, nt, L], F32, tag="simm", name=f"sm{k}")
                    nc.vector.select(simm[:], rearr(vmj[k], nt), sim[:],
                                     rearr(c_negL[:, 0:nt * L], nt))
                    maxv = band_p.tile([128, nt], F32, tag="maxv", name=f"mx{k}")
                    nc.vector.tensor_reduce(out=maxv[:], in_=simm[:],
                                            axis=mybir.AxisListType.X, op=AluOpType.max)
                    eq = band_p.tile([128, nt, L], F32, tag="eq", name=f"eq{k}")
                    nc.vector.tensor_tensor(out=eq[:], in0=simm[:],
                                            in1=bcast_l(maxv[:], nt), op=AluOpType.is_equal)
                    wt = band_p.tile([128, nt, L], F32, tag="wt", name=f"wq{k}")
                    nc.gpsimd.tensor_tensor(out=wt[:], in0=eq[:],
                                            in1=rearr(riota[:, 0:nt * L], nt),
                                            op=AluOpType.mult)
                    mval = band_p.tile([128, nt], F32, tag="mval", name=f"mv{k}")
                    nc.vector.tensor_reduce(out=mval[:], in_=wt[:],
                                            axis=mybir.AxisListType.X, op=AluOpType.max)
                    mvei = band_p.tile([128, 2 * nt], F32, tag="mvei", name=f"me{k}")
                    # end = cconst - mval  (into ei half)
                    nc.gpsimd.tensor_tensor(out=mvei[:, nt:2 * nt], in0=cconst[k],
                                            in1=mval[:], op=AluOpType.subtract)
                    # mv = maxv * rsq     (into mv half)
                    nc.vector.tensor_scalar(out=mvei[:, 0:nt], in0=maxv[:],
                                            scalar1=rsqall[:, k:k + 1], scalar2=None,
                                            op0=AluOpType.mult)
                    # vmi ships inverted (1 = invalid start): overwrite those
                    nc.vector.copy_predicated(mvei[:, 0:nt], vmi[k], c_negL[:, 0:nt])
                    nc.vector.copy_predicated(mvei[:, nt:2 * nt], vmi[k], c_neg1[:, 0:nt])
                    nc.sync.dma_start(
                        bass.AP(mvei_out.tensor, k * 2 * NT_MAX * 128,
                                [[2 * nt, 128], [1, 2 * nt]]), mvei[:])

            dps0, nt0, np0 = emit_phase_a(0)
            emit_drain(0, dps0, nt0, np0)
            dps1, nt1_, np1 = emit_phase_a(1, banded_mid=lambda: emit_banded(0))
            emit_drain(1, dps1, nt1_, np1)
            emit_banded(1)

    nc.compile()
    return nc


def _prep_core(seq, idx, order_c, W, NT, pairs):
    """Host-side input prep for one core."""
    NT_MAX = max(NT)
    LU = sum(NT[k] * L for k in range(4))
    LI = sum(NT[k] for k in range(4))
    p128 = np.arange(128)
    qh = np.zeros((128, NC * 2 * 4), np.float16)
    ql = np.zeros((128, NC * 2 * 4), np.float16)
    qcat = np.zeros((128, 4 * 16), np.float32)
    im = {}
    spans = {}
    for p, (ka, kb) in enumerate(pairs):
        Wp = W[ka] + W[kb]
        hilo = np.full((NC, 128, 2 * Wp), PAD_VAL, np.float16)
        hilo[:, :, Wp:] = 0.0
        for k, wlo in ((ka, 0), (kb, W[ka])):
            b = order_c[k]
            sep0, sep1 = int(idx[b, 0]), int(idx[b, 1])
            span = max(0, sep1 - sep0 - 1)
            spans[k] = span
            x = np.ascontiguousarray(seq[b, sep0 + 1:sep0 + 1 + span, :].T)
            xh = x.astype(np.float16)
            xl = (x - xh.astype(np.float32)).astype(np.float16)
            hilo[:, :, wlo:wlo + span] = xh.reshape(NC, 128, span)
            hilo[:, :, Wp + wlo:Wp + wlo + span] = xl.reshape(NC, 128, span)
            q1 = seq[b, 1, :]
            q2 = seq[b, max(sep0 - 1, 0), :]
            q1h, q2h = q1.astype(np.float16), q2.astype(np.float16)
            q1l = (q1 - q1h.astype(np.float32)).astype(np.float16)
            q2l = (q2 - q2h.astype(np.float32)).astype(np.float16)
            j0 = 0 if k == ka else 2
            for c in range(NC):
                base = (c * 2 + p) * 4 + j0
                sl = slice(c * 128, (c + 1) * 128)
                qh[:, base] = q1h[sl]
                qh[:, base + 1] = q2h[sl]
                ql[:, base] = q1l[sl]
                ql[:, base + 1] = q2l[sl]
            qcat[:, k * 16:k * 16 + 8] = q1.reshape(128, 8, order="F")
            qcat[:, k * 16 + 8:k * 16 + 16] = q2.reshape(128, 8, order="F")
        im[f"hilo{p}"] = np.ascontiguousarray(
            hilo.transpose(1, 0, 2).reshape(128, NC * 2 * Wp))
    vpack = np.zeros((128, LU + LI), np.uint8)
    fpack = np.zeros((128, NT_MAX * L + LI + 4 * 16), np.float32)
    off_u, off_i, off_f = 0, LU, NT_MAX * L
    fpack[:, 0:NT_MAX * L] = np.broadcast_to(
        (L - np.arange(L))[None, None, :], (128, NT_MAX, L)).reshape(128, NT_MAX * L)
    for k in range(4):
        b = order_c[k]
        sep0 = int(idx[b, 0])
        span = spans[k]
        nt = NT[k]
        i_comp = p128[:, None] + 128 * np.arange(nt)[None, :]
        jv = i_comp[:, :, None] + np.arange(L)[None, None, :]
        vpack[:, off_u:off_u + nt * L] = (jv < span).astype(np.uint8).reshape(128, nt * L)
        off_u += nt * L
        # inverted: 1 where INVALID start (for copy_predicated masking)
        vpack[:, off_i:off_i + nt] = (i_comp >= span).astype(np.uint8)
        off_i += nt
        fpack[:, off_f:off_f + nt] = (sep0 + 1 + i_comp + L).astype(np.float32)
        off_f += nt
    fpack[:, off_f:off_f + 4 * 16] = qcat
    im["vpack"] = vpack
    im["fpack"] = fpack
    im["qhl"] = np.concatenate([qh, ql], axis=1)
    im["ones"] = np.ones((128, 2), np.float32)
    return im, spans


def kernel(sequence_outputs, idxs, max_ans_len):
    seq = np.asarray(sequence_outputs, dtype=np.float32)
    idx = np.asarray(idxs).astype(np.int64)
    assert int(max_ans_len) == L and seq.shape == (B, S, H)

    spans_all = np.maximum(idx[:, 1] - idx[:, 0] - 1, 0)
    order = np.argsort(-spans_all, kind="stable")
    W = [max(2, (int(spans_all[order[k * NCORES]]) + 1) & ~1) for k in range(4)]
    NT = [(w + 127) // 128 for w in W]
    pairs = [(0, 3), (1, 2)]

    key = (tuple(W),)
    if key not in _cache:
        _cache[key] = _build(W, NT, pairs)
    nc = _cache[key]

    NT_MAX = max(NT)
    in_maps, span_list = [], []
    for c in range(NCORES):
        order_c = [int(order[k * NCORES + c]) for k in range(4)]
        im, spans = _prep_core(seq, idx, order_c, W, NT, pairs)
        in_maps.append(im)
        span_list.append((order_c, spans))

    res = run_bass_kernel_spmd(nc, in_maps, core_ids=list(range(NCORES))).results

    mv = np.full((B, S), NEG, np.float32)
    ei = np.full((B, S), -1, np.int32)
    for c in range(NCORES):
        order_c, spans = span_list[c]
        for k in range(4):
            b = order_c[k]
            sep0 = int(idx[b, 0])
            span = spans[k]
            if span <= 0:
                continue
            nt = NT[k]
            flat = res[c]["mvei"][k, 0:128 * 2 * nt].reshape(128, 2 * nt)
            mvd = flat[:, 0:nt].T.ravel()
            eid = flat[:, nt:2 * nt].T.ravel()
            mv[b, sep0 + 1:sep0 + 1 + span] = mvd[0:span]
            ei[b, sep0 + 1:sep0 + 1 + span] = np.rint(eid[0:span]).astype(np.int32)
    return mv, ei


# revision 20
# speedup vs baseline: 1.1635x; 1.1458x over previous
"""Trainium2 Bass kernel for the span-search problem (nn_DCR_21285857919673).

Data-parallel over batch: 32 batches / 8 cores = 4 per core. The host ships
seq pre-transposed ([h, token]), compacted to the valid token span
(sep0+1 .. sep1), and split losslessly into an fp16 hi/lo pair (same 4B/elem
of DMA as fp32, but PE fp16 matmuls run 4x faster than fp32). Per core the
4 batches are sorted into width slots and processed as 2 pairs so each
PSUM-accumulating matmul streams two batches' tokens at once.

Per pair, per h-chunk (8 chunks of 128):
  PE:  d1,d2 via 3 fp16 matmul passes (hi*qhi + hi*qlo + lo*qhi) -> fp32 PSUM
       (error ~2^-22: full-fp32 quality for the argmax ties), in 512-col
       bank-aligned pieces; n2 via fp32r ones-matmuls over Act-squared hi
       (n2 only needs ~5e-5 relative accuracy -- it enters through
       sqrt + ratio -- so the tf32-grade fp32r path is safe there).
Then d/n2 rows go to DRAM scratch and the banded window stage
(overlapping-AP gathers, masked max / first-argmax with a Newton-corrected
division) runs per slot, split across DVE/Pool/Act. Pair k's banded stage
is emitted inside pair k+1's chunk loop so it overlaps the next pair's
compute; DMA instruction count is kept low (HWDGE prep is ~0.6us each).
"""
import sys

sys.path.insert(0, "/opt/trn_rl_repo")

import numpy as np

import concourse.bass as bass
import concourse.bacc as bacc
import concourse.bass_isa as bass_isa
import concourse.mybir as mybir
import concourse.tile as tile
from concourse.alu_op_type import AluOpType
from concourse.bass_utils import run_bass_kernel_spmd

F32 = mybir.dt.float32
F32R = mybir.dt.float32r
F16 = mybir.dt.float16
I32 = mybir.dt.int32
U8 = mybir.dt.uint8

B = 32
S = 1024
H = 1024
L = 32
NC = H // 128          # h chunks
NCORES = 8
NEG = -10000.0
PAD_VAL = 0.25

CHUNK_GROUPS = [[0], [1, 2], [3, 4], [5, 6], [7]]

_cache = {}


def _pieces_bank(w):
    """Bank-aligned <=512 pieces of [0, w)."""
    return [(off, min(512, w - off)) for off in range(0, w, 512)]


def _pieces_balanced(w):
    """Even-length pieces of [0, w), each <=512 (fp32r needs even cols,
    and >=256 cols for full rate)."""
    assert w % 2 == 0
    n = max(1, (w + 511) // 512)
    base = (w // n) & ~1
    out = []
    off = 0
    for i in range(n):
        ln = base if i < n - 1 else w - off
        out.append((off, ln))
        off += ln
    assert all(ln % 2 == 0 and ln <= 512 for _, ln in out)
    return out


def _build(W, NT, pairs):
    """W: slot widths [4] (even), NT: ceil(W/128) [4], pairs: [(a, b), ...]."""
    NT_MAX = max(NT)
    SP = NT_MAX * 128 + 64      # scratch row pitch
    LU = sum(NT[k] * L for k in range(4))
    LI = sum(NT[k] for k in range(4))
    nc = bacc.Bacc("TRN2", target_bir_lowering=False, debug=False)

    # partition-major so multi-chunk DMA slices match SBUF iteration order
    hilos = []
    for p, (a, b) in enumerate(pairs):
        Wp = W[a] + W[b]
        hilos.append(nc.dram_tensor(f"hilo{p}", [128, NC * 2 * Wp], F16,
                                    kind="ExternalInput").ap())
    # qhl: qh block then ql block, each [128, NC*2*4]
    qhl_in = nc.dram_tensor("qhl", [128, 2 * NC * 2 * 4], F16, kind="ExternalInput").ap()
    # u8 pack: all vmj (slot-major) then all vmi
    vpack_in = nc.dram_tensor("vpack", [128, LU + LI], U8, kind="ExternalInput").ap()
    # f32 pack: riota | all cconst | qcat
    fpack_in = nc.dram_tensor("fpack", [128, NT_MAX * L + LI + 4 * 16], F32,
                              kind="ExternalInput").ap()
    ones_in = nc.dram_tensor("ones", [128, 2], F32R, kind="ExternalInput").ap()

    mvei_out = nc.dram_tensor("mvei", [4, 2 * NT_MAX * 128], F32, kind="ExternalOutput").ap()
    scratch = nc.dram_tensor("scratch", [4, 3, SP], F32).ap()

    with tile.TileContext(nc) as tc:
        with (
            tc.tile_pool(name="consts", bufs=1) as consts,
            tc.tile_pool(name="seqp", bufs=3) as seqp,
            tc.tile_pool(name="sqp", bufs=3) as sqp,
            tc.tile_pool(name="rows", bufs=2) as rows_p,
            tc.tile_pool(name="band", bufs=2) as band_p,
            tc.tile_pool(name="pd", bufs=1, space="PSUM") as pd,
            tc.tile_pool(name="pn", bufs=1, space="PSUM") as pn,
        ):
            # ---------- pair-0 chunk streams first (PE warmup path) ----------
            pair_tiles = {}

            def emit_chunk_dmas(p, Wp):
                tiles = []
                for g in CHUNK_GROUPS:
                    t = seqp.tile([128, len(g) * 2 * Wp], F16, tag=f"hilo{len(g)}",
                                  name=f"hilo_p{p}_g{g[0]}")
                    nc.sync.dma_start(t[:], hilos[p][:, g[0] * 2 * Wp:(g[-1] + 1) * 2 * Wp])
                    for gi, c in enumerate(g):
                        tiles.append((t, gi))
                return tiles

            Wps = [W[a] + W[b] for a, b in pairs]

            # ---------- consts first: qhl gates the very first matmul ----------
            c_qhl = consts.tile([128, 2 * NC * 2 * 4], F16, tag="qhl")
            nc.sync.dma_start(c_qhl[:], qhl_in)
            c_ones = consts.tile([128, 2], F32R, tag="ones")
            nc.sync.dma_start(c_ones[:], ones_in)
            c_vpack = consts.tile([128, LU + LI], U8, tag="vpack")
            nc.sync.dma_start(c_vpack[:], vpack_in)
            c_fpack = consts.tile([128, NT_MAX * L + LI + 4 * 16], F32, tag="fpack")
            nc.sync.dma_start(c_fpack[:], fpack_in)

            pair_tiles[0] = emit_chunk_dmas(0, Wps[0])
            pair_tiles[1] = emit_chunk_dmas(1, Wps[1])

            # const views
            off_u = [0]
            for k in range(4):
                off_u.append(off_u[-1] + NT[k] * L)
            vmj = [c_vpack[:, off_u[k]:off_u[k + 1]] for k in range(4)]
            off_i = LU
            vmi = []
            for k in range(4):
                vmi.append(c_vpack[:, off_i:off_i + NT[k]])
                off_i += NT[k]
            riota = c_fpack[:, 0:NT_MAX * L]
            off_f = NT_MAX * L
            cconst = []
            for k in range(4):
                cconst.append(c_fpack[:, off_f:off_f + NT[k]])
                off_f += NT[k]
            qcat = c_fpack[:, off_f:off_f + 4 * 16]

            c_negL = consts.tile([128, NT_MAX * L], F32, tag="negL")
            nc.vector.memset(c_negL[:], NEG)
            c_neg1 = consts.tile([128, NT_MAX], F32, tag="neg1")
            nc.vector.memset(c_neg1[:], -1.0)

            qpart = consts.tile([128, 4], F32, tag="qpart")
            qtrash = consts.tile([128, 16], F32, tag="qtrash")
            qn2all = consts.tile([128, 4], F32, tag="qn2all")
            qsq = consts.tile([128, 4], F32, tag="qsq")
            rsqall = consts.tile([128, 4], F32, tag="rsqall")

            def emit_qn2():
                for k in range(4):
                    nc.scalar.activation(qtrash[:], qcat[:, k * 16:(k + 1) * 16],
                                         mybir.ActivationFunctionType.Square,
                                         accum_out=qpart[:, k:k + 1])
                nc.gpsimd.partition_all_reduce(qn2all[:], qpart[:], 128,
                                               bass_isa.ReduceOp.add)
                nc.scalar.sqrt(qsq[:], qn2all[:])
                nc.vector.reciprocal(rsqall[:], qsq[:])

            def bcast_l(ap_col, nt):
                return bass.AP(ap_col.tensor, ap_col.offset,
                               [[ap_col.ap[0][0], 128], [ap_col.ap[-1][0], nt], [0, L]])

            def win_col(tile_, nt):
                a = tile_[:]
                return bass.AP(tile_.tensor, a.offset,
                               [[a.ap[0][0], 128], [a.ap[1][0], nt], [0, L]])

            def rearr(ap_, nt):
                return bass.AP(ap_.tensor, ap_.offset,
                               [[ap_.ap[0][0], 128], [L, nt], [1, L]])

            # ---------- phase A (matmuls/squares) ----------
            def emit_phase_a(p, banded_mid=None):
                ka, kb = pairs[p]
                Wp = Wps[p]
                dpieces = _pieces_bank(Wp)
                npieces = _pieces_balanced(Wp)
                dps = pd.tile([4, Wp], F32, tag="dps", name=f"dps{p}")
                ntile = [pn.tile([2, 512], F32, tag=f"n{j}", name=f"n{j}_{p}")
                         for j in range(len(npieces))]
                sq_tiles = []
                for c in range(NC):
                    t, gi = pair_tiles[p][c]
                    hi_c = t[:, gi * 2 * Wp: gi * 2 * Wp + Wp]
                    lo_c = t[:, gi * 2 * Wp + Wp: (gi + 1) * 2 * Wp]
                    qh_sl = c_qhl[:, (c * 2 + p) * 4:(c * 2 + p) * 4 + 4]
                    ql_sl = c_qhl[:, NC * 8 + (c * 2 + p) * 4:NC * 8 + (c * 2 + p) * 4 + 4]
                    # pass-major so lhsT stays loaded (Ldweights are costly on
                    # the PE sequencer): qh over hi+lo, then ql over hi
                    for off, ln in dpieces:
                        nc.tensor.matmul(dps[:, off:off + ln], lhsT=qh_sl,
                                         rhs=hi_c[:, off:off + ln],
                                         start=(c == 0), stop=False)
                    for off, ln in dpieces:
                        nc.tensor.matmul(dps[:, off:off + ln], lhsT=qh_sl,
                                         rhs=lo_c[:, off:off + ln],
                                         start=False, stop=False)
                    for off, ln in dpieces:
                        nc.tensor.matmul(dps[:, off:off + ln], lhsT=ql_sl,
                                         rhs=hi_c[:, off:off + ln],
                                         start=False, stop=(c == NC - 1))
                    sq_c = sqp.tile([128, Wp], F32R, tag="sq", name=f"sq{p}_{c}")
                    nc.scalar.activation(sq_c[:], hi_c,
                                         mybir.ActivationFunctionType.Square)
                    sq_tiles.append(sq_c)
                    for cc2 in ([c - 1] if c > 0 else []) + ([c] if c == NC - 1 else []):
                        for j, (off, ln) in enumerate(npieces):
                            nc.tensor.matmul(ntile[j][0:1, 0:ln], lhsT=c_ones[:, 0:1],
                                             rhs=sq_tiles[cc2][:, off:off + ln],
                                             start=(cc2 == 0), stop=(cc2 == NC - 1))
                    if p == 0 and c == NC - 1:
                        emit_qn2()
                    if banded_mid is not None and c == 3:
                        banded_mid()
                return dps, ntile, npieces

            # ---------- phase B part 1: drain + scratch ----------
            def emit_drain(p, dps, ntile, npieces):
                ka, kb = pairs[p]
                Wp = Wps[p]
                Wa = W[ka]
                dsb = rows_p.tile([4, Wp + 160], F32, tag="dsb", name=f"dsb{p}")
                nc.scalar.copy(dsb[:, 0:Wp], dps[:])
                n2sb = rows_p.tile([1, Wp + 160], F32, tag="n2sb", name=f"n2sb{p}")
                for j, (off, ln) in enumerate(npieces):
                    nc.vector.tensor_copy(n2sb[0:1, off:off + ln], ntile[j][0:1, 0:ln])
                # scratch rows; width covers the window pad region ([W_k, NT_k*128+32))
                for k, r0, wlo in ((ka, 0, 0), (kb, 2, Wa)):
                    wk = NT[k] * 128 + 32
                    nc.sync.dma_start(
                        bass.AP(scratch.tensor, k * 3 * SP, [[SP, 2], [1, wk]]),
                        dsb[r0:r0 + 2, wlo:wlo + wk])
                    nc.sync.dma_start(
                        bass.AP(scratch.tensor, (k * 3 + 2) * SP, [[1, 1], [1, wk]]),
                        n2sb[0:1, wlo:wlo + wk])

            # ---------- phase B part 2: banded stage for one pair ----------
            def emit_banded(p):
                ka, kb = pairs[p]
                work = []
                for k in (ka, kb):
                    nt = NT[k]
                    soff = k * 3 * SP
                    d1col = band_p.tile([128, nt], F32, tag="d1col", name=f"d1c{k}")
                    nc.sync.dma_start(d1col[:], bass.AP(scratch.tensor, soff,
                                                        [[1, 128], [128, nt]]))
                    d2w = band_p.tile([128, nt, L], F32, tag="d2w", name=f"d2w{k}")
                    nc.sync.dma_start(d2w[:], bass.AP(scratch.tensor, soff + SP,
                                                      [[1, 128], [128, nt], [1, L]]))
                    n2w = band_p.tile([128, nt, L], F32, tag="n2w", name=f"n2w{k}")
                    nc.sync.dma_start(n2w[:], bass.AP(scratch.tensor, soff + 2 * SP,
                                                      [[1, 128], [128, nt], [1, L]]))
                    work.append((k, nt, d1col, d2w, n2w))

                st = {}
                for k, nt, d1col, d2w, n2w in work:
                    numer = band_p.tile([128, nt, L], F32, tag="numer", name=f"nu{k}")
                    nc.gpsimd.tensor_tensor(out=numer[:], in0=d2w[:],
                                            in1=bcast_l(d1col[:], nt), op=AluOpType.add)
                    nsum = band_p.tile([128, nt, L], F32, tag="nsum", name=f"ns{k}")
                    nc.vector.tensor_tensor(out=nsum[:], in0=n2w[:], in1=win_col(n2w, nt),
                                            op=AluOpType.add)
                    st[k] = [numer, nsum]
                for k, nt, d1col, d2w, n2w in work:
                    numer, nsum = st[k]
                    den = band_p.tile([128, nt, L], F32, tag="den", name=f"de{k}")
                    nc.scalar.sqrt(den[:], nsum[:])
                    rec = band_p.tile([128, nt, L], F32, tag="rec", name=f"re{k}")
                    nc.vector.reciprocal(rec[:], den[:])
                    st[k] += [den, rec]
                for k, nt, d1col, d2w, n2w in work:
                    numer, nsum, den, rec = st[k]
                    sim0 = band_p.tile([128, nt, L], F32, tag="sim0", name=f"s0{k}")
                    nc.vector.tensor_tensor(out=sim0[:], in0=numer[:], in1=rec[:],
                                            op=AluOpType.mult)
                    nt1 = band_p.tile([128, nt, L], F32, tag="nt1", name=f"t1{k}")
                    nc.gpsimd.tensor_tensor(out=nt1[:], in0=sim0[:], in1=den[:],
                                            op=AluOpType.mult)
                    nt2 = band_p.tile([128, nt, L], F32, tag="nt2", name=f"t2{k}")
                    nc.gpsimd.tensor_tensor(out=nt2[:], in0=numer[:], in1=nt1[:],
                                            op=AluOpType.subtract)
                    nt3 = band_p.tile([128, nt, L], F32, tag="nt3", name=f"t3{k}")
                    nc.gpsimd.tensor_tensor(out=nt3[:], in0=nt2[:], in1=rec[:],
                                            op=AluOpType.mult)
                    sim = band_p.tile([128, nt, L], F32, tag="sim", name=f"si{k}")
                    nc.gpsimd.tensor_tensor(out=sim[:], in0=sim0[:], in1=nt3[:],
                                            op=AluOpType.add)
                    simm = band_p.tile([128, nt, L], F32, tag="simm", name=f"sm{k}")
                    nc.vector.select(simm[:], rearr(vmj[k], nt), sim[:],
                                     rearr(c_negL[:, 0:nt * L], nt))
                    maxv = band_p.tile([128, nt], F32, tag="maxv", name=f"mx{k}")
                    nc.vector.tensor_reduce(out=maxv[:], in_=simm[:],
                                            axis=mybir.AxisListType.X, op=AluOpType.max)
                    eq = band_p.tile([128, nt, L], F32, tag="eq", name=f"eq{k}")
                    nc.vector.tensor_tensor(out=eq[:], in0=simm[:],
                                            in1=bcast_l(maxv[:], nt), op=AluOpType.is_equal)
                    wt = band_p.tile([128, nt, L], F32, tag="wt", name=f"wq{k}")
                    nc.gpsimd.tensor_tensor(out=wt[:], in0=eq[:],
                                            in1=rearr(riota[:, 0:nt * L], nt),
                                            op=AluOpType.mult)
                    mval = band_p.tile([128, nt], F32, tag="mval", name=f"mv{k}")
                    nc.vector.tensor_reduce(out=mval[:], in_=wt[:],
                                            axis=mybir.AxisListType.X, op=AluOpType.max)
                    mvei = band_p.tile([128, 2 * nt], F32, tag="mvei", name=f"me{k}")
                    # end = cconst - mval  (into ei half)
                    nc.gpsimd.tensor_tensor(out=mvei[:, nt:2 * nt], in0=cconst[k],
                                            in1=mval[:], op=AluOpType.subtract)
                    # mv = maxv * rsq     (into mv half)
                    nc.vector.tensor_scalar(out=mvei[:, 0:nt], in0=maxv[:],
                                            scalar1=rsqall[:, k:k + 1], scalar2=None,
                                            op0=AluOpType.mult)
                    # vmi ships inverted (1 = invalid start): overwrite those
                    nc.vector.copy_predicated(mvei[:, 0:nt], vmi[k], c_negL[:, 0:nt])
                    nc.vector.copy_predicated(mvei[:, nt:2 * nt], vmi[k], c_neg1[:, 0:nt])
                    nc.sync.dma_start(
                        bass.AP(mvei_out.tensor, k * 2 * NT_MAX * 128,
                                [[2 * nt, 128], [1, 2 * nt]]), mvei[:])

            dps0, nt0, np0 = emit_phase_a(0)
            emit_drain(0, dps0, nt0, np0)
            dps1, nt1_, np1 = emit_phase_a(1, banded_mid=lambda: emit_banded(0))
            emit_drain(1, dps1, nt1_, np1)
            emit_banded(1)

    nc.compile()
    return nc


def _prep_core(seq, idx, order_c, W, NT, pairs):
    """Host-side input prep for one core."""
    NT_MAX = max(NT)
    LU = sum(NT[k] * L for k in range(4))
    LI = sum(NT[k] for k in range(4))
    p128 = np.arange(128)
    qh = np.zeros((128, NC * 2 * 4), np.float16)
    ql = np.zeros((128, NC * 2 * 4), np.float16)
    qcat = np.zeros((128, 4 * 16), np.float32)
    im = {}
    spans = {}
    for p, (ka, kb) in enumerate(pairs):
        Wp = W[ka] + W[kb]
        hilo = np.full((NC, 128, 2 * Wp), PAD_VAL, np.float16)
        hilo[:, :, Wp:] = 0.0
        for k, wlo in ((ka, 0), (kb, W[ka])):
            b = order_c[k]
            sep0, sep1 = int(idx[b, 0]), int(idx[b, 1])
            span = max(0, sep1 - sep0 - 1)
            spans[k] = span
            x = np.ascontiguousarray(seq[b, sep0 + 1:sep0 + 1 + span, :].T)
            xh = x.astype(np.float16)
            xl = (x - xh.astype(np.float32)).astype(np.float16)
            hilo[:, :, wlo:wlo + span] = xh.reshape(NC, 128, span)
            hilo[:, :, Wp + wlo:Wp + wlo + span] = xl.reshape(NC, 128, span)
            q1 = seq[b, 1, :]
            q2 = seq[b, max(sep0 - 1, 0), :]
            q1h, q2h = q1.astype(np.float16), q2.astype(np.float16)
            q1l = (q1 - q1h.astype(np.float32)).astype(np.float16)
            q2l = (q2 - q2h.astype(np.float32)).astype(np.float16)
            j0 = 0 if k == ka else 2
            for c in range(NC):
                base = (c * 2 + p) * 4 + j0
                sl = slice(c * 128, (c + 1) * 128)
                qh[:, base] = q1h[sl]
                qh[:, base + 1] = q2h[sl]
                ql[:, base] = q1l[sl]
                ql[:, base + 1] = q2l[sl]
            qcat[:, k * 16:k * 16 + 8] = q1.reshape(128, 8, order="F")
            qcat[:, k * 16 + 8:k * 16 + 16] = q2.reshape(128, 8, order="F")
        im[f"hilo{p}"] = np.ascontiguousarray(
            hilo.transpose(1, 0, 2).reshape(128, NC * 2 * Wp))
    vpack = np.zeros((128, LU + LI), np.uint8)
    fpack = np.zeros((128, NT_MAX * L + LI + 4 * 16), np.float32)
    off_u, off_i, off_f = 0, LU, NT_MAX * L
    fpack[:, 0:NT_MAX * L] = np.broadcast_to(
        (L - np.arange(L))[None, None, :], (128, NT_MAX, L)).reshape(128, NT_MAX * L)
    for k in range(4):
        b = order_c[k]
        sep0 = int(idx[b, 0])
        span = spans[k]
        nt = NT[k]
        i_comp = p128[:, None] + 128 * np.arange(nt)[None, :]
        jv = i_comp[:, :, None] + np.arange(L)[None, None, :]
        vpack[:, off_u:off_u + nt * L] = (jv < span).astype(np.uint8).reshape(128, nt * L)
        off_u += nt * L
        # inverted: 1 where INVALID start (for copy_predicated masking)
        vpack[:, off_i:off_i + nt] = (i_comp >= span).astype(np.uint8)
        off_i += nt
        fpack[:, off_f:off_f + nt] = (sep0 + 1 + i_comp + L).astype(np.float32)
        off_f += nt
    fpack[:, off_f:off_f + 4 * 16] = qcat
    im["vpack"] = vpack
    im["fpack"] = fpack
    im["qhl"] = np.concatenate([qh, ql], axis=1)
    im["ones"] = np.ones((128, 2), np.float32)
    return im, spans


def kernel(sequence_outputs, idxs, max_ans_len):
    seq = np.asarray(sequence_outputs, dtype=np.float32)
    idx = np.asarray(idxs).astype(np.int64)
    assert int(max_ans_len) == L and seq.shape == (B, S, H)

    spans_all = np.maximum(idx[:, 1] - idx[:, 0] - 1, 0)
    order = np.argsort(-spans_all, kind="stable")
    W = [max(2, (int(spans_all[order[k * NCORES]]) + 1) & ~1) for k in range(4)]
    NT = [(w + 127) // 128 for w in W]
    pairs = [(0, 3), (1, 2)]

    key = (tuple(W),)
    if key not in _cache:
        _cache[key] = _build(W, NT, pairs)
    nc = _cache[key]

    NT_MAX = max(NT)
    in_maps, span_list = [], []
    for c in range(NCORES):
        order_c = [int(order[k * NCORES + c]) for k in range(4)]
        im, spans = _prep_core(seq, idx, order_c, W, NT, pairs)
        in_maps.append(im)
        span_list.append((order_c, spans))

    res = run_bass_kernel_spmd(nc, in_maps, core_ids=list(range(NCORES))).results

    mv = np.full((B, S), NEG, np.float32)
    ei = np.full((B, S), -1, np.int32)
    for c in range(NCORES):
        order_c, spans = span_list[c]
        for k in range(4):
            b = order_c[k]
            sep0 = int(idx[b, 0])
            span = spans[k]
            if span <= 0:
                continue
            nt = NT[k]
            flat = res[c]["mvei"][k, 0:128 * 2 * nt].reshape(128, 2 * nt)
            mvd = flat[:, 0:nt].T.ravel()
            eid = flat[:, nt:2 * nt].T.ravel()
            mv[b, sep0 + 1:sep0 + 1 + span] = mvd[0:span]
            ei[b, sep0 + 1:sep0 + 1 + span] = np.rint(eid[0:span]).astype(np.int32)
    return mv, ei


# revision 21
# speedup vs baseline: 1.3710x; 1.1784x over previous
"""Trainium2 Bass kernel for the span-search problem (nn_DCR_21285857919673).

Data-parallel over batch: 32 batches / 8 cores = 4 per core. The host ships
seq pre-transposed ([h, token]), compacted to the valid token span
(sep0+1 .. sep1), and split losslessly into an fp16 hi/lo pair (same 4B/elem
of DMA as fp32, but PE fp16 matmuls run 4x faster than fp32). Batches are
globally sorted by span into 4 width slots (one batch per slot per core) so
the compiled widths are minimal; each core processes its slots
largest-first.

Per slot, per h-chunk (8 chunks of 128):
  PE:  d1,d2 via 3 fp16 matmul passes (hi*qhi + hi*qlo + lo*qhi) -> fp32 PSUM
       (error ~2^-22: full-fp32 quality for the argmax ties), in 512-col
       bank-aligned pieces; n2 via fp32r ones-matmuls over Act-squared hi
       (n2 only needs ~5e-5 relative accuracy -- it enters through
       sqrt + ratio -- so the tf32-grade fp32r path is safe there).
Then d/n2 rows go to DRAM scratch and the banded window stage
(overlapping-AP gathers, masked max / first-argmax) runs, split across
DVE/Pool/Act. Slot k's banded stage is emitted inside slot k+1's chunk loop
so only the last (smallest) slot's banded work trails the matmul phase.
"""
import sys

sys.path.insert(0, "/opt/trn_rl_repo")

import numpy as np

import concourse.bass as bass
import concourse.bacc as bacc
import concourse.bass_isa as bass_isa
import concourse.mybir as mybir
import concourse.tile as tile
from concourse.alu_op_type import AluOpType
from concourse.bass_utils import run_bass_kernel_spmd

F32 = mybir.dt.float32
F32R = mybir.dt.float32r
F16 = mybir.dt.float16
I32 = mybir.dt.int32
U8 = mybir.dt.uint8

B = 32
S = 1024
H = 1024
L = 32
NC = H // 128
NCORES = 8
NEG = -10000.0
PAD_VAL = 0.25

CHUNK_GROUPS = [[0], [1, 2, 3], [4, 5], [6, 7]]

_cache = {}


def _pieces_bank(w):
    return [(off, min(512, w - off)) for off in range(0, w, 512)]


def _pieces_balanced(w):
    """Even-length pieces <=512 (fp32r needs even cols, >=256 for full rate)."""
    assert w % 2 == 0
    n = max(1, (w + 511) // 512)
    base = (w // n) & ~1
    out = []
    off = 0
    for i in range(n):
        ln = base if i < n - 1 else w - off
        out.append((off, ln))
        off += ln
    assert all(ln % 2 == 0 and ln <= 512 for _, ln in out)
    return out


def _build(W, NT):
    """W: slot widths [4] (even, desc), NT: ceil(W/128)."""
    NT_MAX = max(NT)
    SP = NT_MAX * 128 + 64
    LU = sum(NT[k] * L for k in range(4))
    LI = sum(NT[k] for k in range(4))
    nc = bacc.Bacc("TRN2", target_bir_lowering=False, debug=False)

    # partition-major; per chunk: [hi (W) | lo (W)]
    hilos = [nc.dram_tensor(f"hilo{k}", [128, NC * 2 * W[k]], F16,
                            kind="ExternalInput").ap() for k in range(4)]
    # qh block then ql block, each [128, NC*4*2]: (c, slot) -> 2 cols
    qhl_in = nc.dram_tensor("qhl", [128, 2 * NC * 4 * 2], F16, kind="ExternalInput").ap()
    vpack_in = nc.dram_tensor("vpack", [128, LU + LI], U8, kind="ExternalInput").ap()
    fpack_in = nc.dram_tensor("fpack", [128, NT_MAX * L + LI + 4 * 16], F32,
                              kind="ExternalInput").ap()
    ones_in = nc.dram_tensor("ones", [128, 2], F32R, kind="ExternalInput").ap()

    mvei_out = nc.dram_tensor("mvei", [4, 2 * NT_MAX * 128], F32, kind="ExternalOutput").ap()
    scratch = nc.dram_tensor("scratch", [4, 3, SP], F32).ap()

    with tile.TileContext(nc) as tc:
        with (
            tc.tile_pool(name="consts", bufs=1) as consts,
            tc.tile_pool(name="seqp", bufs=3) as seqp,
            tc.tile_pool(name="sqp", bufs=3) as sqp,
            tc.tile_pool(name="rows", bufs=2) as rows_p,
            tc.tile_pool(name="band", bufs=2) as band_p,
            tc.tile_pool(name="pd", bufs=2, space="PSUM") as pd,
            tc.tile_pool(name="pn", bufs=2, space="PSUM") as pn,
        ):
            # qhl gates the first matmul: ship it first (tiny)
            c_qhl = consts.tile([128, 2 * NC * 4 * 2], F16, tag="qhl")
            nc.sync.dma_start(c_qhl[:], qhl_in)
            c_ones = consts.tile([128, 2], F32R, tag="ones")
            nc.sync.dma_start(c_ones[:], ones_in)

            slot_tiles = {}

            def emit_chunk_dmas(k):
                tiles = []
                w2 = 2 * W[k]
                for g in CHUNK_GROUPS:
                    t = seqp.tile([128, len(g) * w2], F16, tag=f"hilo{len(g)}",
                                  name=f"hilo_s{k}_g{g[0]}")
                    nc.sync.dma_start(t[:], hilos[k][:, g[0] * w2:(g[-1] + 1) * w2])
                    for gi, c in enumerate(g):
                        tiles.append((t, gi))
                slot_tiles[k] = tiles

            emit_chunk_dmas(0)

            c_vpack = consts.tile([128, LU + LI], U8, tag="vpack")
            nc.sync.dma_start(c_vpack[:], vpack_in)
            c_fpack = consts.tile([128, NT_MAX * L + LI + 4 * 16], F32, tag="fpack")
            nc.sync.dma_start(c_fpack[:], fpack_in)

            emit_chunk_dmas(1)

            # const views
            off_u = [0]
            for k in range(4):
                off_u.append(off_u[-1] + NT[k] * L)
            vmj = [c_vpack[:, off_u[k]:off_u[k + 1]] for k in range(4)]
            off_i = LU
            vmi = []
            for k in range(4):
                vmi.append(c_vpack[:, off_i:off_i + NT[k]])
                off_i += NT[k]
            riota = c_fpack[:, 0:NT_MAX * L]
            off_f = NT_MAX * L
            cconst = []
            for k in range(4):
                cconst.append(c_fpack[:, off_f:off_f + NT[k]])
                off_f += NT[k]
            qcat = c_fpack[:, off_f:off_f + 4 * 16]

            c_negL = consts.tile([128, NT_MAX * L], F32, tag="negL")
            nc.vector.memset(c_negL[:], NEG)
            c_neg1 = consts.tile([128, NT_MAX], F32, tag="neg1")
            nc.vector.memset(c_neg1[:], -1.0)

            qpart = consts.tile([128, 4], F32, tag="qpart")
            qtrash = consts.tile([128, 16], F32, tag="qtrash")
            qn2all = consts.tile([128, 4], F32, tag="qn2all")
            qsq = consts.tile([128, 4], F32, tag="qsq")
            rsqall = consts.tile([128, 4], F32, tag="rsqall")

            def emit_qn2():
                for k in range(4):
                    nc.scalar.activation(qtrash[:], qcat[:, k * 16:(k + 1) * 16],
                                         mybir.ActivationFunctionType.Square,
                                         accum_out=qpart[:, k:k + 1])
                nc.gpsimd.partition_all_reduce(qn2all[:], qpart[:], 128,
                                               bass_isa.ReduceOp.add)
                nc.scalar.sqrt(qsq[:], qn2all[:])
                nc.vector.reciprocal(rsqall[:], qsq[:])

            def bcast_l(ap_col, nt):
                return bass.AP(ap_col.tensor, ap_col.offset,
                               [[ap_col.ap[0][0], 128], [ap_col.ap[-1][0], nt], [0, L]])

            def win_col(tile_, nt):
                a = tile_[:]
                return bass.AP(tile_.tensor, a.offset,
                               [[a.ap[0][0], 128], [a.ap[1][0], nt], [0, L]])

            def rearr(ap_, nt):
                return bass.AP(ap_.tensor, ap_.offset,
                               [[ap_.ap[0][0], 128], [L, nt], [1, L]])

            def emit_phase_a(k, mid=None):
                w = W[k]
                dpieces = _pieces_bank(w)
                npieces = _pieces_balanced(w)
                dps = pd.tile([2, w], F32, tag="dps", name=f"dps{k}")
                ntile = [pn.tile([1, 512], F32, tag=f"n{j}", name=f"n{j}_{k}")
                         for j in range(len(npieces))]
                sq_tiles = []
                for c in range(NC):
                    t, gi = slot_tiles[k][c]
                    w2 = 2 * w
                    hi_c = t[:, gi * w2: gi * w2 + w]
                    lo_c = t[:, gi * w2 + w: (gi + 1) * w2]
                    qh_sl = c_qhl[:, (c * 4 + k) * 2:(c * 4 + k) * 2 + 2]
                    ql_sl = c_qhl[:, NC * 8 + (c * 4 + k) * 2:NC * 8 + (c * 4 + k) * 2 + 2]
                    for off, ln in dpieces:
                        nc.tensor.matmul(dps[:, off:off + ln], lhsT=qh_sl,
                                         rhs=hi_c[:, off:off + ln],
                                         start=(c == 0), stop=False)
                    for off, ln in dpieces:
                        nc.tensor.matmul(dps[:, off:off + ln], lhsT=qh_sl,
                                         rhs=lo_c[:, off:off + ln],
                                         start=False, stop=False)
                    for off, ln in dpieces:
                        nc.tensor.matmul(dps[:, off:off + ln], lhsT=ql_sl,
                                         rhs=hi_c[:, off:off + ln],
                                         start=False, stop=(c == NC - 1))
                    sq_c = sqp.tile([128, w], F32R, tag="sq", name=f"sq{k}_{c}")
                    nc.scalar.activation(sq_c[:], hi_c,
                                         mybir.ActivationFunctionType.Square)
                    sq_tiles.append(sq_c)
                    for cc2 in ([c - 1] if c > 0 else []) + ([c] if c == NC - 1 else []):
                        for j, (off, ln) in enumerate(npieces):
                            nc.tensor.matmul(ntile[j][0:1, 0:ln], lhsT=c_ones[:, 0:1],
                                             rhs=sq_tiles[cc2][:, off:off + ln],
                                             start=(cc2 == 0), stop=(cc2 == NC - 1))
                    if k == 0 and c == NC - 1:
                        emit_qn2()
                    if mid is not None and c == 2:
                        mid()
                return dps, ntile, npieces

            def emit_drain(k, dps, ntile, npieces):
                w = W[k]
                wk = NT[k] * 128 + 32
                dsb = rows_p.tile([2, wk + 8], F32, tag="dsb", name=f"dsb{k}")
                nc.scalar.copy(dsb[:, 0:w], dps[:])
                n2sb = rows_p.tile([1, wk + 8], F32, tag="n2sb", name=f"n2sb{k}")
                for j, (off, ln) in enumerate(npieces):
                    nc.vector.tensor_copy(n2sb[0:1, off:off + ln], ntile[j][0:1, 0:ln])
                # width covers the window pad region [W, NT*128+32) with
                # whatever finite/NaN garbage the tiles hold -- it is masked.
                nc.sync.dma_start(
                    bass.AP(scratch.tensor, k * 3 * SP, [[SP, 2], [1, wk]]),
                    dsb[:, 0:wk])
                nc.sync.dma_start(
                    bass.AP(scratch.tensor, (k * 3 + 2) * SP, [[1, 1], [1, wk]]),
                    n2sb[0:1, 0:wk])

            def emit_banded(k):
                nt = NT[k]
                soff = k * 3 * SP
                d1col = band_p.tile([128, nt], F32, tag="d1col", name=f"d1c{k}")
                nc.sync.dma_start(d1col[:], bass.AP(scratch.tensor, soff,
                                                    [[1, 128], [128, nt]]))
                d2w = band_p.tile([128, nt, L], F32, tag="d2w", name=f"d2w{k}")
                nc.sync.dma_start(d2w[:], bass.AP(scratch.tensor, soff + SP,
                                                  [[1, 128], [128, nt], [1, L]]))
                n2w = band_p.tile([128, nt, L], F32, tag="n2w", name=f"n2w{k}")
                nc.sync.dma_start(n2w[:], bass.AP(scratch.tensor, soff + 2 * SP,
                                                  [[1, 128], [128, nt], [1, L]]))

                numer = band_p.tile([128, nt, L], F32, tag="numer", name=f"nu{k}")
                nc.gpsimd.tensor_tensor(out=numer[:], in0=d2w[:],
                                        in1=bcast_l(d1col[:], nt), op=AluOpType.add)
                nsum = band_p.tile([128, nt, L], F32, tag="nsum", name=f"ns{k}")
                nc.vector.tensor_tensor(out=nsum[:], in0=n2w[:], in1=win_col(n2w, nt),
                                        op=AluOpType.add)
                den = band_p.tile([128, nt, L], F32, tag="den", name=f"de{k}")
                nc.scalar.sqrt(den[:], nsum[:])
                rec = band_p.tile([128, nt, L], F32, tag="rec", name=f"re{k}")
                nc.vector.reciprocal(rec[:], den[:])
                sim = band_p.tile([128, nt, L], F32, tag="sim", name=f"si{k}")
                nc.vector.tensor_tensor(out=sim[:], in0=numer[:], in1=rec[:],
                                        op=AluOpType.mult)
                simm = band_p.tile([128, nt, L], F32, tag="simm", name=f"sm{k}")
                nc.vector.select(simm[:], rearr(vmj[k], nt), sim[:],
                                 rearr(c_negL[:, 0:nt * L], nt))
                maxv = band_p.tile([128, nt], F32, tag="maxv", name=f"mx{k}")
                nc.vector.tensor_reduce(out=maxv[:], in_=simm[:],
                                        axis=mybir.AxisListType.X, op=AluOpType.max)
                eq = band_p.tile([128, nt, L], F32, tag="eq", name=f"eq{k}")
                nc.vector.tensor_tensor(out=eq[:], in0=simm[:],
                                        in1=bcast_l(maxv[:], nt), op=AluOpType.is_equal)
                wt = band_p.tile([128, nt, L], F32, tag="wt", name=f"wq{k}")
                nc.gpsimd.tensor_tensor(out=wt[:], in0=eq[:],
                                        in1=rearr(riota[:, 0:nt * L], nt),
                                        op=AluOpType.mult)
                mval = band_p.tile([128, nt], F32, tag="mval", name=f"mv{k}")
                nc.vector.tensor_reduce(out=mval[:], in_=wt[:],
                                        axis=mybir.AxisListType.X, op=AluOpType.max)
                mvei = band_p.tile([128, 2 * nt], F32, tag="mvei", name=f"me{k}")
                nc.gpsimd.tensor_tensor(out=mvei[:, nt:2 * nt], in0=cconst[k],
                                        in1=mval[:], op=AluOpType.subtract)
                nc.vector.tensor_scalar(out=mvei[:, 0:nt], in0=maxv[:],
                                        scalar1=rsqall[:, k:k + 1], scalar2=None,
                                        op0=AluOpType.mult)
                # vmi ships inverted (1 = invalid start)
                nc.vector.copy_predicated(mvei[:, 0:nt], vmi[k], c_negL[:, 0:nt])
                nc.vector.copy_predicated(mvei[:, nt:2 * nt], vmi[k], c_neg1[:, 0:nt])
                nc.sync.dma_start(
                    bass.AP(mvei_out.tensor, k * 2 * NT_MAX * 128,
                            [[2 * nt, 128], [1, 2 * nt]]), mvei[:])

            state = {}

            def mk_mid(k):
                def mid():
                    dps, ntile, npieces = state[k]
                    emit_drain(k, dps, ntile, npieces)
                    emit_banded(k)
                return mid

            for k in range(4):
                if k == 2:
                    emit_chunk_dmas(2)  # emitted late to keep SP queue flowing
                if k == 3:
                    emit_chunk_dmas(3)
                state[k] = emit_phase_a(k, mid=mk_mid(k - 1) if k > 0 else None)
            emit_drain(3, *state[3])
            emit_banded(3)

    nc.compile()
    return nc


def _prep_core(seq, idx, order_c, W, NT):
    NT_MAX = max(NT)
    LU = sum(NT[k] * L for k in range(4))
    LI = sum(NT[k] for k in range(4))
    p128 = np.arange(128)
    qh = np.zeros((128, NC * 4 * 2), np.float16)
    ql = np.zeros((128, NC * 4 * 2), np.float16)
    qcat = np.zeros((128, 4 * 16), np.float32)
    im = {}
    spans = {}
    for k in range(4):
        w = W[k]
        b = order_c[k]
        sep0, sep1 = int(idx[b, 0]), int(idx[b, 1])
        span = max(0, sep1 - sep0 - 1)
        spans[k] = span
        hilo = np.full((NC, 128, 2 * w), PAD_VAL, np.float16)
        hilo[:, :, w:] = 0.0
        x = np.ascontiguousarray(seq[b, sep0 + 1:sep0 + 1 + span, :].T)
        xh = x.astype(np.float16)
        xl = (x - xh.astype(np.float32)).astype(np.float16)
        hilo[:, :, 0:span] = xh.reshape(NC, 128, span)
        hilo[:, :, w:w + span] = xl.reshape(NC, 128, span)
        im[f"hilo{k}"] = np.ascontiguousarray(
            hilo.transpose(1, 0, 2).reshape(128, NC * 2 * w))
        q1 = seq[b, 1, :]
        q2 = seq[b, max(sep0 - 1, 0), :]
        q1h, q2h = q1.astype(np.float16), q2.astype(np.float16)
        q1l = (q1 - q1h.astype(np.float32)).astype(np.float16)
        q2l = (q2 - q2h.astype(np.float32)).astype(np.float16)
        for c in range(NC):
            base = (c * 4 + k) * 2
            sl = slice(c * 128, (c + 1) * 128)
            qh[:, base] = q1h[sl]
            qh[:, base + 1] = q2h[sl]
            ql[:, base] = q1l[sl]
            ql[:, base + 1] = q2l[sl]
        qcat[:, k * 16:k * 16 + 8] = q1.reshape(128, 8, order="F")
        qcat[:, k * 16 + 8:k * 16 + 16] = q2.reshape(128, 8, order="F")
    vpack = np.zeros((128, LU + LI), np.uint8)
    fpack = np.zeros((128, NT_MAX * L + LI + 4 * 16), np.float32)
    off_u, off_i, off_f = 0, LU, NT_MAX * L
    fpack[:, 0:NT_MAX * L] = np.broadcast_to(
        (L - np.arange(L))[None, None, :], (128, NT_MAX, L)).reshape(128, NT_MAX * L)
    for k in range(4):
        b = order_c[k]
        sep0 = int(idx[b, 0])
        span = spans[k]
        nt = NT[k]
        i_comp = p128[:, None] + 128 * np.arange(nt)[None, :]
        jv = i_comp[:, :, None] + np.arange(L)[None, None, :]
        vpack[:, off_u:off_u + nt * L] = (jv < span).astype(np.uint8).reshape(128, nt * L)
        off_u += nt * L
        vpack[:, off_i:off_i + nt] = (i_comp >= span).astype(np.uint8)
        off_i += nt
        fpack[:, off_f:off_f + nt] = (sep0 + 1 + i_comp + L).astype(np.float32)
        off_f += nt
    fpack[:, off_f:off_f + 4 * 16] = qcat
    im["vpack"] = vpack
    im["fpack"] = fpack
    im["qhl"] = np.concatenate([qh, ql], axis=1)
    im["ones"] = np.ones((128, 2), np.float32)
    return im, spans


def kernel(sequence_outputs, idxs, max_ans_len):
    seq = np.asarray(sequence_outputs, dtype=np.float32)
    idx = np.asarray(idxs).astype(np.int64)
    assert int(max_ans_len) == L and seq.shape == (B, S, H)

    spans_all = np.maximum(idx[:, 1] - idx[:, 0] - 1, 0)
    order = np.argsort(-spans_all, kind="stable")
    W = [max(2, (int(spans_all[order[k * NCORES]]) + 1) & ~1) for k in range(4)]
    NT = [(w + 127) // 128 for w in W]

    key = (tuple(W),)
    if key not in _cache:
        _cache[key] = _build(W, NT)
    nc = _cache[key]

    NT_MAX = max(NT)
    in_maps, span_list = [], []
    for c in range(NCORES):
        order_c = [int(order[k * NCORES + c]) for k in range(4)]
        im, spans = _prep_core(seq, idx, order_c, W, NT)
        in_maps.append(im)
        span_list.append((order_c, spans))

    res = run_bass_kernel_spmd(nc, in_maps, core_ids=list(range(NCORES))).results

    mv = np.full((B, S), NEG, np.float32)
    ei = np.full((B, S), -1, np.int32)
    for c in range(NCORES):
        order_c, spans = span_list[c]
        for k in range(4):
            b = order_c[k]
            sep0 = int(idx[b, 0])
            span = spans[k]
            if span <= 0:
                continue
            nt = NT[k]
            flat = res[c]["mvei"][k, 0:128 * 2 * nt].reshape(128, 2 * nt)
            mvd = flat[:, 0:nt].T.ravel()
            eid = flat[:, nt:2 * nt].T.ravel()
            mv[b, sep0 + 1:sep0 + 1 + span] = mvd[0:span]
            ei[b, sep0 + 1:sep0 + 1 + span] = np.rint(eid[0:span]).astype(np.int32)
    return mv, ei
